# revision 43
# baseline (speedup 1.0000x reference)
"""Multi-head attention (B=4, S=2048, D=1024, H=16) on 8 Trainium2 NeuronCores.

Sharding: batch (4-way data parallel) x head-group (2-way tensor parallel).
Core c handles batch c//2, heads [8*(c%2), 8*(c%2)+8).  Each core computes a
partial output [S, D] (its heads' contribution through its Wo row-slice); the
host sums the two partials per batch (bf16 partials, f32 sum, /256 to undo
the fp8-friendly weight pre-scaling).

Per-core kernel (all matmuls bf16, fp32 PSUM accumulation).  Design:
  - softmax denominator comes FREE from the EV matmul: V carries a 65th
    column per head holding 0.25, so the EV output row 64 accumulates
    0.25*sum_k(e) across all 16 key chunks in PSUM.  (Replaces the old
    bf16 tree-sum that burned 181us of DVE.)
  - after the last EV chunk the raw [65,512] PSUM tiles are immediately
    evacuated to SBUF f32 (one DVE copy each) so the next block's EV can
    start (single-buffered EV PSUM); reciprocal/broadcast/normalize run
    off the critical path.  Head B's rows DMA-shift to partitions 64..127
    to match attnT's pair layout for the Wo matmuls.
  - normalize: reciprocal_approx_fast on the denominator rows, K=1 ones
    matmuls broadcast 1/denom to 64 rows each, DVE multiplies write attnT.
  - the weights are pre-scaled x8 on the host; the combined 1/(8*64*...)
    = 1/512 with 1/sqrt(depth) folds into the ACT free affine (scale);
    the host divides the summed partials by 256.
  - scores matmuls carry explicit row-tile positions (0,0)/(64,0) +
    skip_group_check: the two K=64 head matmuls stream CONCURRENTLY in the
    PE (1024 q-columns in ~215ns).  Scores for two key chunks are emitted
    back-to-back (both ps_sc buffers) because every switch between the
    row-tiled config and the full-array EV/projection config costs a
    ~100-150ns PE drain.
  - inputs DMA'd in fine pieces (seed slices first); dummy exp at t=0
    triggers the ACT table load during the DMA window; PE warm-up spin
    keeps the HAM clock at 2.4GHz for the seed projections.
  - projection/WO thunks drip into the PE stream between score matmuls via
    the pending/pop pacing machinery; EV + normalize trail the exp stream
    by EVLAG chunks.
"""

import os

os.environ.setdefault("MYCRO_LOCAL_CACHE", "1")

from collections import deque
from contextlib import ExitStack

import numpy as np
import ml_dtypes

import concourse.bacc as bacc
import concourse.mybir as mybir
import concourse.tile as tile

BF = mybir.dt.bfloat16
F32 = mybir.dt.float32
FP8 = mybir.dt.float8e4
BF_NP = ml_dtypes.bfloat16
FP8_NP = ml_dtypes.float8_e4m3
DR = mybir.MatmulPerfMode.DoubleRow

B, S, D, H = 4, 2048, 1024, 16
DEPTH = D // H          # 64
HPC = 8                 # heads per core
FPC = HPC * DEPTH       # 512 features per core
P = 128
CH = D // P             # 8 contraction chunks for the projections
NK = S // P             # 16 key chunks
NQ = S // 512           # 4 q chunks
NPAIR = HPC // 2        # 4 head pairs
EVLAG = 16               # chunks the EV stream trails the exp stream
# weights pre-scaled x8 on host; logits_raw = 512*logits_true
ACT_SCALE = 1.0 / 512.0
ONES_VAL = 0.25         # V ones-column: denom_row = 0.25*sum(e); attnT = 32*attn
OUT_DESCALE = 256.0     # host divides the summed partials by this
# block order: pairs {0,1} across all q chunks, then pairs {2,3}
BLOCK_ORDER = [(qc, pr) for h in ((0, 1), (2, 3)) for qc in range(4)
               for pr in h]

_NC_CACHE = {}


def _emit(ctx: ExitStack, tc, xt_d, wq_d, wk_d, wv_d, wo_d, eb_d, out_d):
    nc = tc.nc
    Exp = mybir.ActivationFunctionType.Exp

    const = ctx.enter_context(tc.tile_pool(name="const", bufs=1))
    wpool = ctx.enter_context(tc.tile_pool(name="wpool", bufs=1))
    xpool = ctx.enter_context(tc.tile_pool(name="xpool", bufs=1))
    qkpool = ctx.enter_context(tc.tile_pool(name="qkpool", bufs=1))
    vpool = ctx.enter_context(tc.tile_pool(name="vpool", bufs=1))
    epool = ctx.enter_context(tc.tile_pool(name="epool", bufs=18))
    atpool = ctx.enter_context(tc.tile_pool(name="atpool", bufs=4))
    stpool = ctx.enter_context(tc.tile_pool(name="stpool", bufs=4))
    smpool = ctx.enter_context(tc.tile_pool(name="smpool", bufs=2))
    rawpool = ctx.enter_context(tc.tile_pool(name="rawpool", bufs=3))
    # PSUM budget (8 banks): scores 2x[128,1024] = 4, EV atA/atB = 2,
    # misc (qkv/wo/rb) 2x[128,512] = 2.
    ps_sc = ctx.enter_context(tc.tile_pool(name="ps_sc", bufs=2, space="PSUM"))
    ps_atA = ctx.enter_context(tc.tile_pool(name="ps_atA", bufs=1,
                                            space="PSUM"))
    ps_atB = ctx.enter_context(tc.tile_pool(name="ps_atB", bufs=1,
                                            space="PSUM"))
    ps_ms = ctx.enter_context(tc.tile_pool(name="ps_ms", bufs=2, space="PSUM"))

    # dummy exp first: walrus puts the ACT table load here, overlapping DMA
    dxp = const.tile([1, 2], F32)
    nc.vector.memset(dxp, 0.0)
    dxe = const.tile([1, 2], BF)
    nc.scalar.activation(dxe, dxp, Exp)

    ones128 = const.tile([P, 1], BF)
    nc.vector.memset(ones128, 1.0)
    ones_rb = const.tile([P, DEPTH], BF)   # sliced [64:65] as K=1 lhsT
    nc.vector.memset(ones_rb, 1.0)
    warm_src = const.tile([P, 512], BF)
    nc.vector.memset(warm_src, 0.0)
    bb_sb = const.tile([P, NK], F32)   # raw bias, chunked [key%128, chunk]
    # PE warm-up spin keeps the PE HAM at 2.4GHz until the seed DMA lands
    for _ in range(12):
        wm = ps_ms.tile([1, 512], F32, tag="ms")
        nc.tensor.matmul(wm, lhsT=ones128, rhs=warm_src, start=True,
                         stop=True)

    wq_sb = wpool.tile([P, NPAIR, CH, P], BF)
    wk_sb = wpool.tile([P, NPAIR, CH, P], BF)
    wv_sb = wpool.tile([P, CH, FPC], BF)
    wo_sb = wpool.tile([P, NPAIR, D], BF)
    # xt is seq-block-major so one piece carries all contraction chunks for
    # one 512-wide seq block
    xt_sb = xpool.tile([P, NQ, CH, 512], BF)

    # ---- input DMAs: seed first, spread over the 3 DMA-capable engines ----
    q_sync = [
        (wq_sb[:, 0:1], wq_d[:, 0:1]),
        (xt_sb[:, 0, 0:3], xt_d[:, 0, 0:3]),
        (wq_sb[:, 1:2], wq_d[:, 1:2]),
        (xt_sb[:, 1], xt_d[:, 1]),
        (xt_sb[:, 3], xt_d[:, 3]),
        (wo_sb[:, 0:2, :], wo_d[:, 0:2, :]),
    ]
    q_scalar = [
        (xt_sb[:, 0, 3:6], xt_d[:, 0, 3:6]),
        (wv_sb[:, 0:4, :], wv_d[:, 0:4, :]),
        (wq_sb[:, 2:4], wq_d[:, 2:4]),
        (xt_sb[:, 2], xt_d[:, 2]),
    ]
    q_gpsimd = [
        (bb_sb, eb_d),
        (wk_sb[:, 0:1], wk_d[:, 0:1]),
        (xt_sb[:, 0, 6:8], xt_d[:, 0, 6:8]),
        (wk_sb[:, 1:2], wk_d[:, 1:2]),
        (wv_sb[:, 4:8, :], wv_d[:, 4:8, :]),
        (wk_sb[:, 2:4], wk_d[:, 2:4]),
        (wo_sb[:, 2:4, :], wo_d[:, 2:4, :]),
    ]
    for eng, pieces in ((nc.sync, q_sync), (nc.scalar, q_scalar),
                        (nc.gpsimd, q_gpsimd)):
        for dst, src in pieces:
            eng.dma_start(out=dst, in_=src)
    dma_engines = (nc.sync, nc.scalar, nc.gpsimd, nc.sync)

    # ---- phase 0: projections, emitted lazily into the attention stream ----
    QT = qkpool.tile([P, NPAIR, S], BF)   # [2 heads x 64 depth, pair, seq]
    KT = qkpool.tile([P, NPAIR, S], BF)
    # V: per head 64 value columns + a 65th holding ONES_VAL (free denom)
    V = vpool.tile([P, NK, HPC, DEPTH + 1], BF)
    nc.vector.memset(V[:, :, :, DEPTH:DEPTH + 1], ONES_VAL)

    def qkt_thunk(w_sb, dst, pair, sc):
        def f(w_sb=w_sb, dst=dst, pair=pair, sc=sc):
            ps = ps_ms.tile([P, 512], F32, tag="ms")
            for c in range(CH):
                nc.tensor.matmul(
                    ps,
                    lhsT=w_sb[:, pair, c, :],
                    rhs=xt_sb[:, sc, c, :],
                    start=(c == 0),
                    stop=(c == CH - 1),
                )
            nc.vector.tensor_copy(dst[:, pair, 512 * sc:512 * (sc + 1)], ps)
        return f

    def v_thunk(sb):
        def f(sb=sb):
            ps = ps_ms.tile([P, 512], F32, tag="ms")
            for c in range(CH):
                nc.tensor.matmul(
                    ps,
                    lhsT=xt_sb[:, sb // 4, c, P * (sb % 4):P * (sb % 4) + P],
                    rhs=wv_sb[:, c, :],
                    start=(c == 0),
                    stop=(c == CH - 1),
                )
            nc.vector.tensor_copy(V[:, sb, :, 0:DEPTH], ps)
        return f

    def wo_thunk(qc, qb, n, attnT):
        def f(qc=qc, qb=qb, n=n, attnT=attnT):
            po = ps_ms.tile([P, 512], F32, tag="ms")
            for pr in range(NPAIR):
                nc.tensor.matmul(
                    po,
                    lhsT=attnT[:, pr, P * qb:P * (qb + 1)],
                    rhs=wo_sb[:, pr, 512 * n:512 * (n + 1)],
                    start=(pr == 0),
                    stop=(pr == NPAIR - 1),
                )
            st = stpool.tile([P, 512], BF, tag="st")
            nc.vector.tensor_copy(st, po)
            qq = 512 * qc + P * qb
            dma_engines[(qb + n) % 2 * 2].dma_start(
                out=out_d[qq:qq + P, 512 * n:512 * (n + 1)], in_=st
            )
        return f

    # seed: exactly what scores(block 0, chunk 0) needs, emitted eagerly
    qkt_thunk(wq_sb, QT, 0, 0)()
    qkt_thunk(wk_sb, KT, 0, 0)()
    emitted = {("QT", 0, 0), ("KT", 0, 0)}
    pending = deque()
    pending.append((("KT", 0, 1), qkt_thunk(wk_sb, KT, 0, 1)))
    pending.append((("V", 0), v_thunk(0)))
    pending.append((("KT", 0, 2), qkt_thunk(wk_sb, KT, 0, 2)))
    pending.append((("V", 1), v_thunk(1)))
    pending.append((("KT", 0, 3), qkt_thunk(wk_sb, KT, 0, 3)))
    pending.append((("V", 2), v_thunk(2)))
    pending.append((("V", 3), v_thunk(3)))
    pending.append((("QT", 1, 0), qkt_thunk(wq_sb, QT, 1, 0)))
    pending.append((("KT", 1, 0), qkt_thunk(wk_sb, KT, 1, 0)))
    for sb in range(4, 6):
        pending.append((("V", sb), v_thunk(sb)))
    pending.append((("KT", 1, 1), qkt_thunk(wk_sb, KT, 1, 1)))
    for sb in range(6, 8):
        pending.append((("V", sb), v_thunk(sb)))
    pending.append((("KT", 1, 2), qkt_thunk(wk_sb, KT, 1, 2)))
    for sb in range(8, 10):
        pending.append((("V", sb), v_thunk(sb)))
    pending.append((("KT", 1, 3), qkt_thunk(wk_sb, KT, 1, 3)))
    for sb in range(10, 12):
        pending.append((("V", sb), v_thunk(sb)))
    pending.append((("QT", 0, 1), qkt_thunk(wq_sb, QT, 0, 1)))
    for sb in range(12, 14):
        pending.append((("V", sb), v_thunk(sb)))
    pending.append((("QT", 1, 1), qkt_thunk(wq_sb, QT, 1, 1)))
    for sb in range(14, 16):
        pending.append((("V", sb), v_thunk(sb)))
    # second-half projections held back until block 4 (scheduler hoisting)
    pending_late = deque()
    for qq in range(2, NQ):
        pending_late.append((("QT", 0, qq), qkt_thunk(wq_sb, QT, 0, qq)))
        pending_late.append((("QT", 1, qq), qkt_thunk(wq_sb, QT, 1, qq)))
    for pp in (2, 3):
        for sc in range(NQ):
            pending_late.append(
                (("KT", pp, sc), qkt_thunk(wk_sb, KT, pp, sc)))
        pending_late.append((("QT", pp, 0), qkt_thunk(wq_sb, QT, pp, 0)))
    for qq in range(1, NQ):
        pending_late.append((("QT", 2, qq), qkt_thunk(wq_sb, QT, 2, qq)))
        pending_late.append((("QT", 3, qq), qkt_thunk(wq_sb, QT, 3, qq)))

    def pop_one():
        if pending:
            key, fn = pending.popleft()
            fn()
            emitted.add(key)

    def need(key):
        while key not in emitted and pending:
            pop_one()
        while key not in emitted and pending_late:
            k2, f2 = pending_late.popleft()
            f2()
            emitted.add(k2)

    # ---- phases 1+2: attention, EV lagged behind the exp stream ----
    ev_queue = deque()

    def make_ev_step(bctx, g):
        def f(bctx=bctx, g=g):
            pair, qc = bctx["pair"], bctx["qc"]
            hA, hB = 2 * pair, 2 * pair + 1
            e = bctx["e"]
            need(("V", g))
            if g == 0:
                atA_new = ps_atA.tile([P, 512], F32, tag="atA")
                atB_new = ps_atB.tile([P, 512], F32, tag="atB")
                bctx["atA"], bctx["atB"] = atA_new, atB_new
            atA, atB = bctx["atA"], bctx["atB"]
            nc.tensor.matmul(
                atA[0:DEPTH + 1, :],
                lhsT=V[:, g, hA, :],
                rhs=e[g][:, 0:512],
                start=(g == 0), stop=(g == NK - 1),
                skip_group_check=True,
            )
            nc.tensor.matmul(
                atB[0:DEPTH + 1, :],
                lhsT=V[:, g, hB, :],
                rhs=e[g][:, 512:1024],
                start=(g == 0), stop=(g == NK - 1),
                skip_group_check=True,
            )
            if g == NK - 1:
                # evacuate raw EV + denominator rows; frees atA/atB so the
                # next block's EV can start (ps_at pools are single-buffered)
                raw = rawpool.tile([P, 1536], F32, tag="raw")
                nc.vector.tensor_copy(raw[0:DEPTH + 1, 0:512],
                                      atA[0:DEPTH + 1, :])
                nc.vector.tensor_copy(raw[0:DEPTH + 1, 512:1024],
                                      atB[0:DEPTH + 1, :])
                # denominator row hops to partition 0 first (it gates the
                # recip -> broadcast chain; the bulkier B-shift only gates
                # the final multiply), both on the quiet gpsimd DMA queue.
                # The partition-0 hop exists because the custom DVE
                # reciprocal mis-executes on non-zero base partitions and
                # sub-dimensioned APs.
                den0 = smpool.tile([1, 1024], F32, tag="den0")
                nc.gpsimd.dma_start(out=den0,
                                    in_=raw[DEPTH:DEPTH + 1, 0:1024])
                # head B's 64 value rows shift to partitions 64..127 (SBUF->
                # SBUF DMA) so the normalize lands in attnT's pair layout
                nc.gpsimd.dma_start(out=raw[DEPTH:P, 1024:1536],
                                    in_=raw[0:DEPTH, 512:1024])
                rcp = smpool.tile([1, 1024], F32, tag="rcp")
                nc.vector.reciprocal_approx_fast(rcp, den0)
                rcb = smpool.tile([1, 1024], BF, tag="rcb")
                nc.vector.tensor_copy(rcb, rcp)
                # the K=1 broadcast matmul + normalize muls go to the pending
                # queue: popped at PE pace a few chunks later, the recip
                # chain is long done, so the ms PSUM slot is held ~1us
                # instead of gating on a DMA round-trip.  The final block
                # runs it inline (nothing left to overlap with).
                nt = norm_thunk(qc, pair, raw, rcb, bctx["attnT"])
                if bctx.get("last"):
                    nt()
                else:
                    pending.append((("NORM", qc, pair), nt))
        return f

    def norm_thunk(qc, pair, raw, rcb, attnT):
        def f(qc=qc, pair=pair, raw=raw, rcb=rcb, attnT=attnT):
            rb = ps_ms.tile([P, 512], F32, tag="ms")
            nc.tensor.matmul(
                rb[0:DEPTH, :],
                lhsT=ones_rb[0:1, :],
                rhs=rcb[:, 0:512],
                start=True, stop=True,
                tile_position=(0, 0),
                skip_group_check=True,
            )
            nc.tensor.matmul(
                rb[DEPTH:P, :],
                lhsT=ones_rb[0:1, :],
                rhs=rcb[:, 512:1024],
                start=True, stop=True,
                tile_position=(0, DEPTH),
                skip_group_check=True,
            )
            nc.vector.tensor_mul(
                attnT[0:DEPTH, pair, :], raw[0:DEPTH, 0:512],
                rb[0:DEPTH, :])
            nc.vector.tensor_mul(
                attnT[DEPTH:P, pair, :], raw[DEPTH:P, 1024:1536],
                rb[DEPTH:P, :])
            if pair == NPAIR - 1:
                for qb in range(4):
                    for n in range(2):
                        pending.append((
                            ("WO", qc, qb, n),
                            wo_thunk(qc, qb, n, attnT),
                        ))
        return f

    attnT_tiles = {}
    for bi, (qc, pair) in enumerate(BLOCK_ORDER):
        q0 = 512 * qc
        lag = 1 if bi == len(BLOCK_ORDER) - 1 else EVLAG
        if 4 <= bi <= 7:
            for _ in range(4):
                if pending_late:
                    pending.append(pending_late.popleft())
        if bi == 8:
            pending.extend(pending_late)
            pending_late.clear()
        if qc not in attnT_tiles:
            # [64 depth, head, seq]: fp8 lhsT for the DoubleRow WO matmuls
            attnT = atpool.tile([P, NPAIR, 512], BF, tag="attnT")
            attnT_tiles[qc] = attnT
        attnT = attnT_tiles[qc]
        bctx = {"pair": pair, "qc": qc, "attnT": attnT, "e": [None] * NK,
                "last": bi == len(BLOCK_ORDER) - 1}
        for g2, nburst in ((0, 2), (2, 2), (4, 2), (6, 2), (8, 2), (10, 2),
                           (12, 2), (14, 2)):
            # scores for up to THREE chunks back-to-back: the row-tiled score
            # pairs and the full-array EV/projection matmuls force a PE
            # mode-switch drain (~150ns) at every config change, so batching
            # several chunks' scores amortizes it
            npop = nburst if pending else 0
            if bi == 0 and g2 == 0:
                npop = 0
            for _ in range(npop):
                pop_one()
            if g2 % 4 < 2:
                need(("KT", pair, g2 // 4))
            if g2 + nburst > 4 * (g2 // 4 + 1):
                need(("KT", pair, g2 // 4 + 1))
            if g2 == 0:
                need(("QT", pair, qc))
            sc_ts = []
            for g in range(g2, g2 + nburst):
                k0 = P * g
                sc_t = ps_sc.tile([P, 1024], F32, tag="sc")
                sc_ts.append(sc_t)
                nc.tensor.matmul(
                    sc_t[:, 0:512],
                    lhsT=KT[0:DEPTH, pair, k0:k0 + P],
                    rhs=QT[0:DEPTH, pair, q0:q0 + 512],
                    start=True, stop=True,
                    tile_position=(0, 0), skip_group_check=True,
                )
                nc.tensor.matmul(
                    sc_t[:, 512:1024],
                    lhsT=KT[DEPTH:P, pair, k0:k0 + P],
                    rhs=QT[DEPTH:P, pair, q0:q0 + 512],
                    start=True, stop=True,
                    tile_position=(DEPTH, 0), skip_group_check=True,
                )
            for g, sc_t in zip(range(g2, g2 + nburst), sc_ts):
                e_t = epool.tile([P, 1024], BF, tag="e")
                # scale folds 1/sqrt(depth) and the x8 weight pre-scaling;
                # bias is per partition (= key): one [P,1] AP for both heads
                nc.scalar.activation(e_t, sc_t, Exp, bias=bb_sb[:, g:g + 1],
                                     scale=ACT_SCALE)
                bctx["e"][g] = e_t
                ev_queue.append(make_ev_step(bctx, g))
                while len(ev_queue) > lag:
                    ev_queue.popleft()()
    while ev_queue:
        ev_queue.popleft()()
    while pending:
        pop_one()


def _build():
    nc = bacc.Bacc("TRN2", target_bir_lowering=False, debug=False)
    xt = nc.dram_tensor("xt", [P, NQ, CH, 512], BF, kind="ExternalInput").ap()
    wq = nc.dram_tensor("wq", [P, NPAIR, CH, P], BF,
                        kind="ExternalInput").ap()
    wk = nc.dram_tensor("wk", [P, NPAIR, CH, P], BF,
                        kind="ExternalInput").ap()
    wv = nc.dram_tensor("wv", [P, CH, FPC], BF, kind="ExternalInput").ap()
    wo = nc.dram_tensor("wo", [P, NPAIR, D], BF, kind="ExternalInput").ap()
    eb = nc.dram_tensor("eb", [P, NK], F32, kind="ExternalInput").ap()
    out = nc.dram_tensor("out", [S, D], BF, kind="ExternalOutput").ap()
    with tile.TileContext(nc) as tc:
        with ExitStack() as ctx:
            _emit(ctx, tc, xt, wq, wk, wv, wo, eb, out)
    nc.compile()
    return nc


def get_nc():
    if "nc" not in _NC_CACHE:
        _NC_CACHE["nc"] = _build()
    return _NC_CACHE["nc"]


def _in_maps(x, bias, Wq, Wk, Wv, Wo):
    x = np.asarray(x, dtype=np.float32)
    bias = np.asarray(bias, dtype=np.float32)
    # x8 pre-scale keeps fp8e4m3 weights in normal range (sigma 1/32 -> 1/4)
    Wq = np.asarray(Wq, dtype=np.float32) * 8.0
    Wk = np.asarray(Wk, dtype=np.float32) * 8.0
    Wv = np.asarray(Wv, dtype=np.float32) * 8.0
    Wo = np.asarray(Wo, dtype=np.float32) * 8.0
    maps = []

    for core in range(8):
        b, grp = core // 2, core % 2
        cols = slice(FPC * grp, FPC * (grp + 1))
        # [D, S] -> [P part, NQ seq-block, CH chunk, 512]
        xt = np.ascontiguousarray(
            x[b].T.astype(BF_NP).reshape(CH, P, NQ, 512).transpose(1, 2, 0, 3)
        )
        # [D, FPC] -> [128 part, pair, chunk, 128]
        wq = np.ascontiguousarray(
            Wq[:, cols].astype(BF_NP).reshape(CH, P, NPAIR, P)
            .transpose(1, 2, 0, 3)
        )
        wk = np.ascontiguousarray(
            Wk[:, cols].astype(BF_NP).reshape(CH, P, NPAIR, P)
            .transpose(1, 2, 0, 3)
        )
        # [D, FPC] -> [128 part, chunk, 512]
        wv = np.ascontiguousarray(
            Wv[:, cols].astype(BF_NP).reshape(CH, P, FPC).swapaxes(0, 1)
        )
        # [FPC, D] -> [128 part = pair-of-heads depth, pair, 1024]
        wo = np.ascontiguousarray(
            Wo[cols, :].astype(BF_NP).reshape(NPAIR, P, D).swapaxes(0, 1)
        )
        eb = np.ascontiguousarray(
            bias[b, 0, 0].astype(np.float32).reshape(NK, P).T
        )  # raw bias, [128 = key%128, 16 = key chunk]
        maps.append(
            {"xt": xt, "wq": wq, "wk": wk, "wv": wv, "wo": wo, "eb": eb}
        )
    return maps


def _get_exec():
    """Cached jitted SPMD executable mirroring bass2jax.run_bass_via_pjrt,
    without donation (our kernel writes every output element) so repeated
    calls can reuse persistent device buffers for timing."""
    if "exec" in _NC_CACHE:
        return _NC_CACHE["exec"]
    import jax
    import concourse.mybir as _mybir
    from concourse.bass2jax import (
        _bass_exec_p,
        install_neuronx_cc_hook,
        partition_id_tensor,
    )
    from jax.experimental.shard_map import shard_map
    from jax.sharding import Mesh, NamedSharding, PartitionSpec

    install_neuronx_cc_hook()
    nc = get_nc()
    n_cores = 8
    part_name = nc.partition_id_tensor.name if nc.partition_id_tensor else None
    in_names, out_names, out_avals = [], [], []
    for alloc in nc.m.functions[0].allocations:
        if not isinstance(alloc, _mybir.MemoryLocationSet):
            continue
        name = alloc.memorylocations[0].name
        if alloc.kind == "ExternalInput":
            if name != part_name:
                in_names.append(name)
        elif alloc.kind == "ExternalOutput":
            out_names.append(name)
            out_avals.append(
                jax.core.ShapedArray(
                    tuple(alloc.tensor_shape), _mybir.dt.np(alloc.dtype)
                )
            )
    n_params = len(in_names)
    all_names = in_names + out_names
    if part_name is not None:
        all_names = all_names + [part_name]

    def _body(*args):
        operands = list(args)
        if part_name is not None:
            operands.append(partition_id_tensor())
        return tuple(
            _bass_exec_p.bind(
                *operands,
                out_avals=tuple(out_avals),
                in_names=tuple(all_names),
                out_names=tuple(out_names),
                lowering_input_output_aliases=(),
                sim_require_finite=True,
                sim_require_nnan=True,
                nc=nc,
            )
        )

    devices = jax.devices()[:n_cores]
    mesh = Mesh(np.asarray(devices), ("core",))
    nshard = NamedSharding(mesh, PartitionSpec("core"))
    sharded = jax.jit(
        shard_map(
            _body,
            mesh=mesh,
            in_specs=(PartitionSpec("core"),) * (n_params + len(out_names)),
            out_specs=(PartitionSpec("core"),) * len(out_names),
            check_rep=False,
        ),
        keep_unused=True,
    )
    zeros = [
        jax.device_put(
            np.zeros((n_cores * a.shape[0], *a.shape[1:]), a.dtype), nshard
        )
        for a in out_avals
    ]
    _NC_CACHE["exec"] = (sharded, in_names, out_names, out_avals, nshard, zeros)
    return _NC_CACHE["exec"]


def _execute(maps):
    import jax

    sharded, in_names, out_names, out_avals, nshard, zeros = _get_exec()
    concat_in = [
        jax.device_put(
            np.concatenate([np.asarray(m[name]) for m in maps], axis=0), nshard
        )
        for name in in_names
    ]
    outs = sharded(*concat_in, *zeros)
    return concat_in, outs, out_names, out_avals


def run(x, bias, Wq, Wk, Wv, Wo, trace=False):
    """Returns (full_output [B,S,D] f32, per-core outs)."""
    maps = _in_maps(x, bias, Wq, Wk, Wv, Wo)
    _, outs, out_names, out_avals = _execute(maps)
    per_core = np.asarray(outs[out_names.index("out")]).reshape(8, S, D)
    full = np.empty((B, S, D), dtype=np.float32)
    for b in range(B):
        full[b] = (
            per_core[2 * b].astype(np.float32)
            + per_core[2 * b + 1].astype(np.float32)
        ) / OUT_DESCALE
    return full, per_core


def bench(x, bias, Wq, Wk, Wv, Wo, iters=20):
    """Amortized per-execution wall time (ns) over pipelined dispatches."""
    import jax
    import time

    maps = _in_maps(x, bias, Wq, Wk, Wv, Wo)
    sharded, in_names, out_names, out_avals, nshard, zeros = _get_exec()
    concat_in = [
        jax.device_put(
            np.concatenate([np.asarray(m[name]) for m in maps], axis=0), nshard
        )
        for name in in_names
    ]
    outs = sharded(*concat_in, *zeros)  # warmup / compile
    jax.block_until_ready(outs)
    t0 = time.perf_counter()
    for _ in range(iters):
        outs = sharded(*concat_in, *zeros)
    jax.block_until_ready(outs)
    dt = (time.perf_counter() - t0) / iters
    return int(dt * 1e9)


def kernel(x, bias, Wq, Wk, Wv, Wo):
    return run(x, bias, Wq, Wk, Wv, Wo)[0]
